# revision 1
# baseline (speedup 1.0000x reference)
"""Trainium2 Bass kernel for the ConsistencyLoss problem.

Inputs: semantic_pred (B,N) int32, instance_masks (B,M,N) f32, depth (B,N) f32
with B=16, M=32, N=65536 (H=W=256), C=27 classes. Outputs the scalar tuple
(l_uniform, l_boundary, l_dbc, total).

Sharding: pure data-parallel over batch; 2 batches per core on 8 cores. Each
core emits 6 partial sums per batch; the host combines the 4 scalars.

Per-core pipeline (per batch):
  * class histograms: instance_masks are stream-transposed on the vector
    engine (32x32 blocks) so the pixel index lands on partitions; the 8 sem
    segments ride a separate int32 transpose tile whose transposed columns
    feed a broadcast is_equal against an iota row (the one-hot); the PE
    accumulates hist[m,c] over 512 K=128 matmul chunks in PSUM.
  * boundary/depth losses: (256,256) views as (128, 2x256) tiles with
    row-shifted copies for partition-direction gradients; per-partition sums
    are folded across partitions with a ones-vector matmul at the end.
"""

import os

os.environ.setdefault("MYCRO_LOCAL_CACHE", "1")

import numpy as np
from contextlib import ExitStack

B, M, N, C = 16, 32, 65536, 27
H = W = 256
NCORES = 8
BPC = B // NCORES          # batches per core
FS = 2048                  # free size per transpose super-tile
G = 4                      # partition groups (of 32) in the stream transpose
ST = N // (G * FS)         # super-tiles per batch = 8
KC = FS // 32              # matmul chunks per super-tile = 64
NSTAT = 12                 # 6 partial sums x 2 batches

LAST_EXEC_NS = None

_compiled = None


def _build():
    import concourse.tile as tile
    from concourse import bacc, mybir

    f32 = mybir.dt.float32
    i32 = mybir.dt.int32
    bf16 = mybir.dt.bfloat16
    OP = mybir.AluOpType
    AX = mybir.AxisListType
    AF = mybir.ActivationFunctionType

    nc = bacc.Bacc("TRN2", target_bir_lowering=False, debug=False,
                   enable_asserts=False, num_swdge_queues=4)
    sem_d = nc.dram_tensor("sem", [BPC, N], i32, kind="ExternalInput")
    masks_d = nc.dram_tensor("masks", [BPC, M, N], f32, kind="ExternalInput")
    depth_d = nc.dram_tensor("depth", [BPC, N], f32, kind="ExternalInput")
    out_d = nc.dram_tensor("partials", [NSTAT], f32, kind="ExternalOutput")

    with tile.TileContext(nc) as tc, ExitStack() as ctx:
        pconst = ctx.enter_context(tc.tile_pool(name="const", bufs=1))
        pin = ctx.enter_context(tc.tile_pool(name="maskin", bufs=3))
        pinb = ctx.enter_context(tc.tile_pool(name="maskinb", bufs=3))
        ptr = ctx.enter_context(tc.tile_pool(name="maskT", bufs=3))
        poh = ctx.enter_context(tc.tile_pool(name="ohp", bufs=3))
        psx = ctx.enter_context(tc.tile_pool(name="semxp", bufs=2))
        pbnd = ctx.enter_context(tc.tile_pool(name="bnd", bufs=1))
        psm = ctx.enter_context(tc.tile_pool(name="small", bufs=2))
        pps = ctx.enter_context(tc.tile_pool(name="psum", bufs=2, space="PSUM"))

        iota = pconst.tile([128, C], i32, tag="iota")
        nc.gpsimd.iota(iota[:], pattern=[[1, C]], base=0, channel_multiplier=0)
        iotab = pconst.tile([128, C], bf16, tag="iotab")
        nc.scalar.copy(iotab[:], iota[:])
        iotarep_i = pconst.tile([128, KC * C], i32, tag="iotarep_i")
        nc.gpsimd.iota(iotarep_i[:], pattern=[[0, KC], [1, C]], base=0,
                       channel_multiplier=0)
        iotarep = pconst.tile([128, KC * C], bf16, tag="iotarep")
        nc.scalar.copy(iotarep[:], iotarep_i[:])
        stats = pconst.tile([128, NSTAT], f32, tag="stats")
        nc.vector.memset(stats[:], 0.0)
        ones = pconst.tile([128, 1], f32, tag="ones")
        nc.vector.memset(ones[:], 1.0)
        bias_ln = pconst.tile([128, 1], f32, tag="bias_ln")
        nc.vector.memset(bias_ln[:], 1e-10)
        bias_sq = pconst.tile([128, 1], f32, tag="bias_sq")
        nc.vector.memset(bias_sq[:], 1e-24)
        zerob = pconst.tile([128, 1], f32, tag="zerob")
        nc.vector.memset(zerob[:], 0.0)
        c09 = pconst.tile([128, 1], f32, tag="c09")
        nc.vector.memset(c09[:], 0.09)

        # ---- boundary + depth losses ----
        def btile(tag, dt=f32):
            return pbnd.tile([128, BPC * 512], dt, tag=tag, name=tag)

        semn = btile("semn", i32)
        sems = btile("sems", i32)
        m0n = btile("m0n")
        m0s = btile("m0s")
        dn = btile("dn")
        ds_ = btile("ds_")

        def load2d(nat, sh, src2d, b):
            o = b * 512
            nc.sync.dma_start(nat[:, o:o + 512],
                              src2d.rearrange("(t p) w -> p t w", p=128))
            nc.sync.dma_start(sh[0:1, o:o + 256], src2d[0:1, :])
            nc.sync.dma_start(sh[1:128, o:o + 256], src2d[0:127, :])
            nc.sync.dma_start(sh[:, o + 256:o + 512], src2d[127:255, :])

        for b in range(BPC):
            load2d(semn, sems, sem_d.ap()[b].rearrange("(h w) -> h w", w=W), b)
            load2d(m0n, m0s, masks_d.ap()[b, 0].rearrange("(h w) -> h w", w=W), b)
            load2d(dn, ds_, depth_d.ap()[b].rearrange("(h w) -> h w", w=W), b)

        def blk(t):
            return t[:].rearrange("p (blk w) -> p blk w", w=256)

        eqx = btile("eqx")
        nc.vector.tensor_tensor(blk(eqx)[:, :, 1:], blk(semn)[:, :, 1:],
                                blk(semn)[:, :, :255], op=OP.is_equal)
        nc.gpsimd.memset(blk(eqx)[:, :, 0:1], 1.0)
        eqy = btile("eqy")
        nc.vector.tensor_tensor(eqy[:], semn[:], sems[:], op=OP.is_equal)
        nb = btile("nb")
        nc.gpsimd.tensor_tensor(nb[:], eqx[:], eqy[:], op=OP.mult)

        mgx = btile("mgx")
        nc.gpsimd.tensor_tensor(blk(mgx)[:, :, 1:], blk(m0n)[:, :, 1:],
                                blk(m0n)[:, :, :255], op=OP.subtract)
        nc.gpsimd.memset(blk(mgx)[:, :, 0:1], 0.0)
        mgy = btile("mgy")
        nc.gpsimd.tensor_tensor(mgy[:], m0n[:], m0s[:], op=OP.subtract)
        sqmx = btile("sqmx")
        nc.scalar.activation(sqmx[:], mgx[:], AF.Square, bias=zerob[:, 0:1])
        sqmy = btile("sqmy")
        nc.scalar.activation(sqmy[:], mgy[:], AF.Square, bias=zerob[:, 0:1])
        sqmm = btile("sqmm")
        nc.vector.tensor_tensor(sqmm[:], sqmx[:], sqmy[:], op=OP.max)
        ib = btile("ib")
        nc.vector.tensor_tensor(ib[:], sqmm[:],
                                c09[:, 0:1].broadcast_to([128, BPC * 512]),
                                op=OP.is_gt)

        dgx = btile("dgx")
        nc.gpsimd.tensor_tensor(blk(dgx)[:, :, 1:], blk(dn)[:, :, 1:],
                                blk(dn)[:, :, :255], op=OP.subtract)
        nc.gpsimd.memset(blk(dgx)[:, :, 0:1], 0.0)
        dgy = btile("dgy")
        nc.gpsimd.tensor_tensor(dgy[:], dn[:], ds_[:], op=OP.subtract)
        sqx = btile("sqx")
        nc.scalar.activation(sqx[:], dgx[:], AF.Square, bias=zerob[:, 0:1])
        sqy = btile("sqy")
        nc.scalar.activation(sqy[:], dgy[:], AF.Square, bias=zerob[:, 0:1])
        s2 = btile("s2")
        nc.gpsimd.tensor_tensor(s2[:], sqx[:], sqy[:], op=OP.add)
        db = btile("db")
        nc.scalar.activation(db[:], s2[:], AF.Sqrt, bias=bias_sq[:, 0:1])
        u3 = btile("u3")
        nc.scalar.activation(u3[:], db[:], AF.Square, bias=zerob[:, 0:1], scale=float(np.sqrt(3.0)))

        pnbib = btile("pnbib")
        nc.gpsimd.tensor_tensor(pnbib[:], nb[:], ib[:], op=OP.mult)
        pnbdb = btile("pnbdb")
        nc.gpsimd.tensor_tensor(pnbdb[:], nb[:], db[:], op=OP.mult)
        pnbu = btile("pnbu")
        nc.gpsimd.tensor_tensor(pnbu[:], nb[:], u3[:], op=OP.mult)
        for b in range(BPC):
            h = slice(b * 512, (b + 1) * 512)
            # stats cols: 6b+0 S_ib, +1 S_nb, +2 S_nbib, +3 S_nbdb, +4 S_nb3db2
            nc.vector.tensor_reduce(stats[:, 6 * b + 0:6 * b + 1], ib[:, h],
                                    axis=AX.X, op=OP.add)
            nc.vector.tensor_reduce(stats[:, 6 * b + 1:6 * b + 2], nb[:, h],
                                    axis=AX.X, op=OP.add)
            nc.vector.tensor_reduce(stats[:, 6 * b + 2:6 * b + 3], pnbib[:, h],
                                    axis=AX.X, op=OP.add)
            nc.vector.tensor_reduce(stats[:, 6 * b + 3:6 * b + 4], pnbdb[:, h],
                                    axis=AX.X, op=OP.add)
            nc.vector.tensor_reduce(stats[:, 6 * b + 4:6 * b + 5], pnbu[:, h],
                                    axis=AX.X, op=OP.add)

        # ---- histogram matmuls ----
        do_hist = not bool(int(os.environ.get("KERNEL_SKIP_HIST", "0")))
        do_bnd = not bool(int(os.environ.get("KERNEL_SKIP_BND", "0")))
        for b in range(BPC if do_hist else 0):
            # tile-X: rows (g, j<ST) hold sem[b, 8192*j + 2048*g : +2048];
            # transposed, col (32k + j) partition (32g+i) = sem at
            # n = 8192*j + 2048*g + 32*k + i  -> the one-hot source.
            tx = psx.tile([128, FS], i32, tag="tx")
            nc.vector.memset(tx[:], 0)
            sem_gsf = sem_d.ap()[b].rearrange("(st g f) -> g st f", st=ST, g=G)
            for g in range(G):
                nc.sync.dma_start(tx[32 * g:32 * g + ST, :], sem_gsf[g])
            txb = psx.tile([128, FS], bf16, tag="txb")
            nc.scalar.copy(txb[:], tx[:])
            txTb = psx.tile([128, FS], bf16, tag="txTb")
            nc.vector.transpose(txTb[:], txb[:])
            txT3 = txTb[:].rearrange("p (k j) -> p k j", j=32)

            hist_ps = pps.tile([32, C], f32, tag="hist")
            for st in range(ST):
                tin = pin.tile([128, FS], f32)
                for g, eng in ((0, nc.sync), (1, nc.gpsimd),
                               (2, nc.scalar), (3, nc.gpsimd)):
                    eng.dma_start(
                        tin[32 * g:32 * (g + 1), :],
                        masks_d.ap()[b, :, st * G * FS + g * FS:
                                     st * G * FS + (g + 1) * FS])
                tinb = pinb.tile([128, FS], bf16)
                nc.scalar.copy(tinb[:], tin[:])
                tT = ptr.tile([128, FS], bf16)
                nc.vector.transpose(tT[:], tinb[:])

                oh = poh.tile([128, KC * C], bf16)
                nc.vector.tensor_tensor(
                    oh[:].rearrange("p (k c) -> p k c", c=C),
                    txT3[:, :, st:st + 1].broadcast_to([128, KC, C]),
                    iotarep[:].rearrange("p (k c) -> p k c", c=C),
                    op=OP.is_equal,
                )
                for k in range(KC):
                    nc.tensor.matmul(
                        hist_ps[:],
                        tT[:, 32 * k:32 * (k + 1)],
                        oh[:, C * k:C * (k + 1)],
                        start=(st == 0 and k == 0),
                        stop=(st == ST - 1 and k == KC - 1),
                    )

            # ---- entropy from hist ----
            hist = psm.tile([32, C], f32, tag="hist_sb")
            nc.scalar.copy(hist[:], hist_ps[:])
            ms0 = psm.tile([32, 1], f32, tag="ms0")
            nc.vector.tensor_reduce(ms0[:], hist[:], axis=AX.X, op=OP.add)
            ms = psm.tile([32, 1], f32, tag="ms")
            nc.vector.tensor_scalar(ms[:], ms0[:], 1e-6, None, op0=OP.add)
            rec = psm.tile([32, 1], f32, tag="rec")
            nc.vector.reciprocal(rec[:], ms[:])
            pr = psm.tile([32, C], f32, tag="pr")
            nc.vector.tensor_scalar(pr[:], hist[:], rec[:, 0:1], None, op0=OP.mult)
            ql = psm.tile([32, C], f32, tag="ql")
            nc.scalar.activation(ql[:], pr[:], AF.Ln, bias=bias_ln[0:32, 0:1])
            escr = psm.tile([32, C], f32, tag="escr")
            nc.vector.tensor_tensor(escr[:], pr[:], ql[:], op=OP.mult)
            ent = psm.tile([32, 1], f32, tag="ent")
            nc.vector.tensor_reduce(ent[:], escr[:], axis=AX.X, op=OP.add)
            nc.vector.tensor_scalar(stats[0:32, 6 * b + 5:6 * b + 6], ent[:],
                                    -1.0, None, op0=OP.mult)

        # ---- cross-partition reduction + output ----
        red_ps = pps.tile([1, NSTAT], f32, tag="red")
        nc.tensor.matmul(red_ps[:], ones[:], stats[:], start=True, stop=True)
        red = pconst.tile([1, NSTAT], f32, tag="redsb")
        nc.scalar.copy(red[:], red_ps[:])
        nc.sync.dma_start(out_d.ap().rearrange("(a b) -> a b", a=1), red[:])

    nc.compile()
    return nc


def _get_nc():
    global _compiled
    if _compiled is None:
        _compiled = _build()
    return _compiled


def _combine(stats):
    """stats: (NCORES, 12) -> (l_uniform, l_boundary, l_dbc, total) fp32."""
    per_b = stats.reshape(B, 6).astype(np.float64)
    s_ib, s_nb, s_nbib, s_nbdb, s_nbu, ent = per_b.T
    inter = s_ib - s_nbib
    union = float(N) - s_nb + s_nbib + 1e-8
    l_boundary = 1.0 - np.mean(inter / union)
    l_uniform = ent.sum() / (B * M + 1e-8)
    l_dbc = (s_nbdb + s_nbu).sum() / (B * N)
    total = 0.3 * l_uniform + 0.2 * l_boundary + 0.2 * l_dbc
    return (np.float32(l_uniform), np.float32(l_boundary),
            np.float32(l_dbc), np.float32(total))


def kernel(semantic_pred, instance_masks, depth, spatial_h=H, spatial_w=W):
    global LAST_EXEC_NS
    from concourse.bass_utils import run_bass_kernel_spmd

    sem = np.ascontiguousarray(np.asarray(semantic_pred, dtype=np.int32))
    masks = np.ascontiguousarray(np.asarray(instance_masks, dtype=np.float32))
    dep = np.ascontiguousarray(np.asarray(depth, dtype=np.float32))

    nc = _get_nc()
    in_maps = [
        {"sem": sem[c * BPC:(c + 1) * BPC],
         "masks": masks[c * BPC:(c + 1) * BPC],
         "depth": dep[c * BPC:(c + 1) * BPC]}
        for c in range(NCORES)
    ]
    trace = bool(int(os.environ.get("KERNEL_TRACE", "0")))
    res = run_bass_kernel_spmd(nc, in_maps, list(range(NCORES)), trace=trace)
    LAST_EXEC_NS = res.exec_time_ns
    stats = np.stack([res.results[c]["partials"] for c in range(NCORES)])
    return _combine(stats)

